# revision 12
# baseline (speedup 1.0000x reference)
"""Deformable conv 3x3 (B=4, C=256, H=W=64, Cout=256) on 8 trn2 NeuronCores.

Sharding: data-parallel — core i handles batch i//2, output-row half i%2
(32 rows = 2048 output positions per core); weight replicated.

Per-core device pipeline (v3, all shapes hardcoded for this problem):
  1. Host precomputes a zero-padded "bilinear basis" image per batch:
     for each padded pixel p=(y,x): [A, C, B, E] x 256ch fp16 where
     A=x[y,x], C=x[y+1,x]-A, B=x[y,x+1]-A, E=x[y+1,x+1]-x[y+1,x]-x[y,x+1]+A.
     Bilinear sample == (A + dx*B) + dy*(C + dx*E), with zero padding
     reproducing the reference's out-of-image masking.
  2. dma_gather (SWDGE): batched — per (jb, tap-triple) gather 1536 basis
     rows (2KB each) -> R [128 j, 12, 1024] fp16, round-robin over 4
     SWDGE queues so descriptor prep overlaps transfers.
  3. Combine split across engines per (tap, jc):
       u = [B,E] * dx        (Scalar engine activation, per-partition scale)
       w = [A,C] + u         (DVE tensor_tensor, 2x fp16)
       g = w_lo + dy * w_hi  (DVE scalar_tensor_tensor, or Scalar mul +
                              DVE add for some taps to balance load)
  4. Transpose G -> rhs [c_kk, j]: PE identity-matmul transposes (PSUM,
     then Scalar copy to SBUF) for most chunks; DMA-XBAR transposes
     (sync-engine dma_start_transpose, straight to SBUF) for some chunks
     to offload the PE.
  5. GEMM: out[o, j] = sum_{c,kk} W[(kk,c), o] * rhs[(kk,c), j], fp32 PSUM,
     K = 2304 (18 chunks), M = 256 (2 chunks), N = 512 per jblock.

kernel(x, offset, weight) takes full fp32 inputs, returns [4,256,64,64] fp32.
"""
import numpy as np
from contextlib import ExitStack

import concourse.bass as bass
import concourse.bacc as bacc
import concourse.tile as tile
from concourse import mybir
from concourse.bass_utils import run_bass_kernel_spmd

# ---------------------------------------------------------------- constants
B, C, H, W = 4, 256, 64, 64
COUT = 256
K = 3
KK = 9
NCORES = 8
ROWS = 32              # output rows per core
J = ROWS * W           # 2048 output positions per core
JBLK = 4               # jblocks
JB = J // JBLK         # 512
JC = JB // 128         # 4
KCH = (C * KK) // 128  # 18 contraction chunks
MCH = COUT // 128      # 2
PADM = 8               # padding margin (covers |offset| < 7)
HP = H + 2 * PADM      # 80
WP = W + 2 * PADM      # 80

TPG = 3                # taps per gather
NGJB = KK // TPG       # gathers per jblock
NIDXG = TPG * JB       # indices per gather (1536)
NCOLG = NIDXG // 16    # idx columns per gather
NQ = 1                 # SWDGE queues

# which taps route step-3's dy-multiply through the Scalar engine
OP3_SCALAR_TAPS = frozenset((1, 4, 7))
# which (kc % 3) values route the transpose through the DMA XBAR
XBAR_KC_MOD = frozenset()

DT = mybir.dt.float16
NPDT = np.float16
F32 = mybir.dt.float32

mult = mybir.AluOpType.mult
add = mybir.AluOpType.add
ACT_COPY = mybir.ActivationFunctionType.Copy


# ---------------------------------------------------------------- host prep
def _make_basis_layout(xb):
    """xb [C,H,W] fp32 -> L [HP*WP, 4*C] fp16 basis rows [A, C, B, E]."""
    xp = np.zeros((HP, WP, C), np.float32)
    xp[PADM:PADM + H, PADM:PADM + W] = xb.transpose(1, 2, 0)
    out = np.zeros((HP, WP, 4, C), np.float32)
    a = xp[:-1, :-1]
    out[:-1, :-1, 0] = a                                # A
    out[:-1, :-1, 1] = xp[1:, :-1] - a                  # C (dy term)
    out[:-1, :-1, 2] = xp[:-1, 1:] - a                  # B (dx term)
    out[:-1, :-1, 3] = xp[1:, 1:] - xp[1:, :-1] - xp[:-1, 1:] + a  # E
    return out.reshape(HP * WP, 4 * C).astype(NPDT)


def _make_idx_w(offset_b, h0):
    """-> idx [KK, J] int16 (padded-grid row), w [KK, J, 2] fp32 (dx, dy)."""
    off = offset_b.reshape(KK, 2, H, W)
    ho = np.arange(h0, h0 + ROWS, dtype=np.float32)
    wo = np.arange(W, dtype=np.float32)
    ky = np.repeat(np.arange(K, dtype=np.float32), K)
    kx = np.tile(np.arange(K, dtype=np.float32), K)
    py = ho[None, :, None] + ky[:, None, None] - 1.0 + off[:, 0, h0:h0 + ROWS, :]
    px = wo[None, None, :] + kx[:, None, None] - 1.0 + off[:, 1, h0:h0 + ROWS, :]
    y0f = np.floor(py)
    x0f = np.floor(px)
    dy = (py - y0f).astype(np.float32)
    dx = (px - x0f).astype(np.float32)
    yi = np.clip(y0f.astype(np.int64) + PADM, 0, HP - 2)
    xi = np.clip(x0f.astype(np.int64) + PADM, 0, WP - 2)
    idx = (yi * WP + xi).astype(np.int16)
    w = np.stack([dx, dy], axis=-1)
    return idx.reshape(KK, J), w.reshape(KK, J, 2)


def _pack_idx(idx):
    """[KK, J] -> [128, JBLK*NGJB*NCOLG] int16; gather g=(jb, tg) covers
    taps tg*TPG..+TPG over jblock jb; idx i at [i%16, i//16] within the
    gather's NCOLG-column slice, replicated to 8 groups of 16 partitions."""
    out = np.zeros((16, JBLK * NGJB * NCOLG), np.int16)
    for jb in range(JBLK):
        for tg in range(NGJB):
            g = jb * NGJB + tg
            v = idx[tg * TPG:(tg + 1) * TPG, jb * JB:(jb + 1) * JB].reshape(-1)
            out[:, g * NCOLG:(g + 1) * NCOLG] = v.reshape(NCOLG, 16).T
    return np.tile(out, (8, 1))


def _pack_w(w):
    """[KK, J, 2] -> [128, KK*JBLK*JC*2] fp32; col ((t*JBLK+jb)*JC+jc)*2+s."""
    a = w.reshape(KK, JBLK, JC, 128, 2)
    return np.ascontiguousarray(
        a.transpose(3, 0, 1, 2, 4).reshape(128, KK * JBLK * JC * 2))


def _pack_weight(weight):
    """[COUT, C, 3, 3] fp32 -> [128, KCH*COUT] fp16; K-order kk*C+c,
    lhsT tile (kc, m) at cols [kc*COUT + m*128, +128)."""
    wm = weight.reshape(COUT, C, KK).transpose(2, 1, 0).reshape(KK * C, COUT)
    wm = wm.reshape(KCH, 128, COUT).transpose(1, 0, 2).reshape(128, KCH * COUT)
    return np.ascontiguousarray(wm).astype(NPDT)


# ---------------------------------------------------------------- program
_PROG = None


def _build_program():
    nc = bacc.Bacc(
        "TRN2",
        target_bir_lowering=False,
        debug=False,
        enable_asserts=False,
        num_devices=NCORES,
        num_swdge_queues=NQ,
    )
    L_t = nc.dram_tensor("xbasis", [HP * WP, 4 * C], DT, kind="ExternalInput")
    WL_t = nc.dram_tensor("wmat", [128, KCH * COUT], DT, kind="ExternalInput")
    IDX_t = nc.dram_tensor("idx", [128, JBLK * NGJB * NCOLG], mybir.dt.int16,
                           kind="ExternalInput")
    WSL_t = nc.dram_tensor("wslot", [128, KK * JBLK * JC * 2], F32,
                           kind="ExternalInput")
    ID_t = nc.dram_tensor("ident", [128, 128], DT, kind="ExternalInput")
    OUT_t = nc.dram_tensor("out", [COUT, J], F32, kind="ExternalOutput")
    out_ap = OUT_t.ap()

    src_ap = bass.AP(L_t, 0, [[4 * C, HP * WP], [1, 4 * C]])

    with tile.TileContext(nc) as tc, ExitStack() as ctx:
        const = ctx.enter_context(tc.tile_pool(name="const", bufs=1))
        rpool = ctx.enter_context(tc.tile_pool(name="r", bufs=3))
        upool = ctx.enter_context(tc.tile_pool(name="u", bufs=4))
        wpool = ctx.enter_context(tc.tile_pool(name="w", bufs=4))
        vpool = ctx.enter_context(tc.tile_pool(name="v", bufs=4))
        gpool = ctx.enter_context(tc.tile_pool(name="g", bufs=2))
        rhspool = ctx.enter_context(tc.tile_pool(name="rhs", bufs=2))
        outpool = ctx.enter_context(tc.tile_pool(name="osb", bufs=2))
        pst = ctx.enter_context(tc.tile_pool(name="pst", bufs=4, space="PSUM"))
        psm = ctx.enter_context(tc.tile_pool(name="psm", bufs=4, space="PSUM"))

        # idx slices load first (per-gather) so gather 0 starts immediately;
        # the big weight matrix loads last (first needed ~40us in).
        idx_sb = const.tile([128, JBLK * NGJB * NCOLG], mybir.dt.int16)
        nc.scalar.dma_start(idx_sb[:], IDX_t.ap())
        wsl_sb = const.tile([128, KK * JBLK * JC * 2], F32)
        nc.sync.dma_start(wsl_sb[:], WSL_t.ap())
        id_sb = const.tile([128, 128], DT)
        nc.sync.dma_start(id_sb[:], ID_t.ap())
        w_sb = const.tile([128, KCH * COUT], DT)
        nc.sync.dma_start(w_sb[:], WL_t.ap())

        reg_full = nc.gpsimd.to_reg(NIDXG)
        reg_2jb = nc.gpsimd.to_reg(2 * JB)
        reg_1jb = nc.gpsimd.to_reg(JB)

        for jb in range(JBLK):
            gt = gpool.tile([128, JC, KK * C], DT)
            for tg in range(NGJB):
                g = jb * NGJB + tg
                last = (jb == JBLK - 1 and tg == NGJB - 1)
                if not last:
                    r = rpool.tile([128, TPG * JC, 4 * C], DT, tag="r")
                    nc.gpsimd.dma_gather(
                        r[:],
                        src_ap,
                        idx_sb[:, g * NCOLG:(g + 1) * NCOLG],
                        NIDXG,
                        reg_full,
                        4 * C,
                        queue_num=g % NQ,
                        single_packet=False,
                    )
                else:
                    # split the final gather 2/3 + 1/3 so its DMA/compute
                    # tail after the descgen train is shorter
                    r = rpool.tile([128, TPG * JC, 4 * C], DT, tag="r")
                    n1 = 2 * JB
                    c0 = g * NCOLG
                    nc.gpsimd.dma_gather(
                        r[:, 0:2 * JC, :],
                        src_ap,
                        idx_sb[:, c0:c0 + n1 // 16],
                        n1,
                        reg_2jb,
                        4 * C,
                        queue_num=g % NQ,
                        single_packet=False,
                    )
                    nc.gpsimd.dma_gather(
                        r[:, 2 * JC:3 * JC, :],
                        src_ap,
                        idx_sb[:, c0 + n1 // 16:(g + 1) * NCOLG],
                        JB,
                        reg_1jb,
                        4 * C,
                        queue_num=g % NQ,
                        single_packet=False,
                    )
                for trel in range(TPG):
                    t = tg * TPG + trel
                    for jc in range(JC):
                        slot = trel * JC + jc
                        cw = ((t * JBLK + jb) * JC + jc) * 2
                        dxs = wsl_sb[:, cw:cw + 1]
                        dys = wsl_sb[:, cw + 1:cw + 2]
                        # w = [A + dx*B, C + dx*E]  (DVE fused STT)
                        w = wpool.tile([128, 2 * C], DT, tag="w")
                        nc.vector.scalar_tensor_tensor(
                            w[:], r[:, slot, 2 * C:4 * C], dxs,
                            r[:, slot, 0:2 * C], mult, add)
                        gslc = gt[:, jc, t * C:(t + 1) * C]
                        if t == KK - 1:
                            # final tap: keep op3 on DVE (fused STT) so the
                            # pipeline tail has one less cross-engine hop
                            nc.vector.scalar_tensor_tensor(
                                gslc, w[:, C:2 * C], dys, w[:, 0:C],
                                mult, add)
                        else:
                            # v = dy * w_hi (Scalar), g = w_lo + v (DVE 2x)
                            v = vpool.tile([128, C], DT, tag="v")
                            nc.scalar.activation(v[:], w[:, C:2 * C],
                                                 ACT_COPY, scale=dys)
                            nc.vector.tensor_add(gslc, w[:, 0:C], v[:])

            rhs = rhspool.tile([128, KCH, JB], DT)
            for kc in range(KCH):
                if kc % 3 in XBAR_KC_MOD:
                    for jc in range(JC):
                        nc.sync.dma_start_transpose(
                            rhs[:, kc, jc * 128:(jc + 1) * 128],
                            gt[:, jc, kc * 128:(kc + 1) * 128])
                else:
                    ps = pst.tile([128, JB], DT)
                    for jc in range(JC):
                        nc.tensor.transpose(ps[:, jc * 128:(jc + 1) * 128],
                                            gt[:, jc, kc * 128:(kc + 1) * 128],
                                            id_sb[:])
                    nc.scalar.copy(rhs[:, kc, :], ps[:])

            for m in range(MCH):
                pso = psm.tile([128, JB], F32)
                for kc in range(KCH):
                    nc.tensor.matmul(
                        pso[:],
                        w_sb[:, kc * COUT + m * 128:kc * COUT + (m + 1) * 128],
                        rhs[:, kc, :],
                        start=(kc == 0),
                        stop=(kc == KCH - 1),
                    )
                osb = outpool.tile([128, JB], F32)
                nc.vector.tensor_copy(osb[:], pso[:])
                nc.sync.dma_start(
                    out_ap[m * 128:(m + 1) * 128, jb * JB:(jb + 1) * JB],
                    osb[:])

    nc.compile()
    return nc


def _get_program():
    global _PROG
    if _PROG is None:
        _PROG = _build_program()
    return _PROG


# ---------------------------------------------------------------- entry
def make_in_maps(x, offset, weight):
    x = np.asarray(x, np.float32)
    offset = np.asarray(offset, np.float32)
    weight = np.asarray(weight, np.float32)
    WL = _pack_weight(weight)
    ident = np.eye(128, dtype=NPDT)
    basis = [_make_basis_layout(x[b]) for b in range(B)]
    maps = []
    for core in range(NCORES):
        b, half = core // 2, core % 2
        idx, w = _make_idx_w(offset[b], half * ROWS)
        maps.append({
            "xbasis": basis[b],
            "idx": _pack_idx(idx),
            "wslot": _pack_w(w),
            "wmat": WL,
            "ident": ident,
        })
    return maps


def assemble(results):
    full = np.zeros((B, COUT, H, W), np.float32)
    for core in range(NCORES):
        b, half = core // 2, core % 2
        h0 = half * ROWS
        full[b, :, h0:h0 + ROWS, :] = \
            np.asarray(results[core]["out"]).reshape(COUT, ROWS, W)
    return full


def kernel(x, offset, weight):
    nc = _get_program()
    in_maps = make_in_maps(x, offset, weight)
    res = run_bass_kernel_spmd(nc, in_maps, list(range(NCORES)))
    return assemble(res.results)


# revision 13
# speedup vs baseline: 1.1879x; 1.1879x over previous
"""Deformable conv 3x3 (B=4, C=256, H=W=64, Cout=256) on 8 trn2 NeuronCores.

Sharding: data-parallel — core i handles batch i//2, output-row half i%2
(32 rows = 2048 output positions per core); weight replicated.

Per-core device pipeline (v3, all shapes hardcoded for this problem):
  1. Host precomputes a zero-padded "bilinear basis" image per batch:
     for each padded pixel p=(y,x): [A, C, B, E] x 256ch fp16 where
     A=x[y,x], C=x[y+1,x]-A, B=x[y,x+1]-A, E=x[y+1,x+1]-x[y+1,x]-x[y,x+1]+A.
     Bilinear sample == (A + dx*B) + dy*(C + dx*E), with zero padding
     reproducing the reference's out-of-image masking.
  2. dma_gather (SWDGE): batched — per (jb, tap-triple) gather 1536 basis
     rows (2KB each) -> R [128 j, 12, 1024] fp16, round-robin over 4
     SWDGE queues so descriptor prep overlaps transfers.
  3. Combine split across engines per (tap, jc):
       u = [B,E] * dx        (Scalar engine activation, per-partition scale)
       w = [A,C] + u         (DVE tensor_tensor, 2x fp16)
       g = w_lo + dy * w_hi  (DVE scalar_tensor_tensor, or Scalar mul +
                              DVE add for some taps to balance load)
  4. Transpose G -> rhs [c_kk, j]: PE identity-matmul transposes (PSUM,
     then Scalar copy to SBUF) for most chunks; DMA-XBAR transposes
     (sync-engine dma_start_transpose, straight to SBUF) for some chunks
     to offload the PE.
  5. GEMM: out[o, j] = sum_{c,kk} W[(kk,c), o] * rhs[(kk,c), j], fp32 PSUM,
     K = 2304 (18 chunks), M = 256 (2 chunks), N = 512 per jblock.

kernel(x, offset, weight) takes full fp32 inputs, returns [4,256,64,64] fp32.
"""
import numpy as np
from contextlib import ExitStack

import concourse.bass as bass
import concourse.bacc as bacc
import concourse.tile as tile
from concourse import mybir
from concourse.bass_utils import run_bass_kernel_spmd

# ---------------------------------------------------------------- constants
B, C, H, W = 4, 256, 64, 64
COUT = 256
K = 3
KK = 9
NCORES = 8
ROWS = 32              # output rows per core
J = ROWS * W           # 2048 output positions per core
JBLK = 4               # jblocks
JB = J // JBLK         # 512
JC = JB // 128         # 4
KCH = (C * KK) // 128  # 18 contraction chunks
MCH = COUT // 128      # 2
PADM = 8               # padding margin (covers |offset| < 7)
HP = H + 2 * PADM      # 80
WP = W + 2 * PADM      # 80

TPG = 3                # taps per gather
NGJB = KK // TPG       # gathers per jblock
NIDXG = TPG * JB       # indices per gather (1536)
NCOLG = NIDXG // 16    # idx columns per gather
NQ = 1                 # SWDGE queues

# which taps route step-3's dy-multiply through the Scalar engine
OP3_SCALAR_TAPS = frozenset((1, 4, 7))
# which (kc % 3) values route the transpose through the DMA XBAR
XBAR_KC_MOD = frozenset()

DT = mybir.dt.float16
NPDT = np.float16
F32 = mybir.dt.float32

mult = mybir.AluOpType.mult
add = mybir.AluOpType.add
ACT_COPY = mybir.ActivationFunctionType.Copy


# ---------------------------------------------------------------- host prep
def _make_basis_layout(xb):
    """xb [C,H,W] fp32 -> L [HP*WP, 4*C] fp16 basis rows [A, C, B, E]."""
    xp = np.zeros((HP, WP, C), np.float32)
    xp[PADM:PADM + H, PADM:PADM + W] = xb.transpose(1, 2, 0)
    out = np.zeros((HP, WP, 4, C), np.float32)
    a = xp[:-1, :-1]
    out[:-1, :-1, 0] = a                                # A
    out[:-1, :-1, 1] = xp[1:, :-1] - a                  # C (dy term)
    out[:-1, :-1, 2] = xp[:-1, 1:] - a                  # B (dx term)
    out[:-1, :-1, 3] = xp[1:, 1:] - xp[1:, :-1] - xp[:-1, 1:] + a  # E
    return out.reshape(HP * WP, 4 * C).astype(NPDT)


def _make_idx_w(offset_b, h0):
    """-> idx [KK, J] int16 (padded-grid row), w [KK, J, 2] fp32 (dx, dy)."""
    off = offset_b.reshape(KK, 2, H, W)
    ho = np.arange(h0, h0 + ROWS, dtype=np.float32)
    wo = np.arange(W, dtype=np.float32)
    ky = np.repeat(np.arange(K, dtype=np.float32), K)
    kx = np.tile(np.arange(K, dtype=np.float32), K)
    py = ho[None, :, None] + ky[:, None, None] - 1.0 + off[:, 0, h0:h0 + ROWS, :]
    px = wo[None, None, :] + kx[:, None, None] - 1.0 + off[:, 1, h0:h0 + ROWS, :]
    y0f = np.floor(py)
    x0f = np.floor(px)
    dy = (py - y0f).astype(np.float32)
    dx = (px - x0f).astype(np.float32)
    yi = np.clip(y0f.astype(np.int64) + PADM, 0, HP - 2)
    xi = np.clip(x0f.astype(np.int64) + PADM, 0, WP - 2)
    idx = (yi * WP + xi).astype(np.int16)
    w = np.stack([dx, dy], axis=-1)
    return idx.reshape(KK, J), w.reshape(KK, J, 2)


def _pack_idx(idx):
    """[KK, J] -> [128, JBLK*NGJB*NCOLG] int16; gather g=(jb, tg) covers
    taps tg*TPG..+TPG over jblock jb; idx i at [i%16, i//16] within the
    gather's NCOLG-column slice, replicated to 8 groups of 16 partitions."""
    out = np.zeros((16, JBLK * NGJB * NCOLG), np.int16)
    for jb in range(JBLK):
        for tg in range(NGJB):
            g = jb * NGJB + tg
            v = idx[tg * TPG:(tg + 1) * TPG, jb * JB:(jb + 1) * JB].reshape(-1)
            out[:, g * NCOLG:(g + 1) * NCOLG] = v.reshape(NCOLG, 16).T
    return np.tile(out, (8, 1))


def _pack_w(w):
    """[KK, J, 2] -> [128, KK*JBLK*JC*2] fp32; col ((t*JBLK+jb)*JC+jc)*2+s."""
    a = w.reshape(KK, JBLK, JC, 128, 2)
    return np.ascontiguousarray(
        a.transpose(3, 0, 1, 2, 4).reshape(128, KK * JBLK * JC * 2))


def _pack_weight(weight):
    """[COUT, C, 3, 3] fp32 -> [128, KCH*COUT] fp16; K-order kk*C+c,
    lhsT tile (kc, m) at cols [kc*COUT + m*128, +128)."""
    wm = weight.reshape(COUT, C, KK).transpose(2, 1, 0).reshape(KK * C, COUT)
    wm = wm.reshape(KCH, 128, COUT).transpose(1, 0, 2).reshape(128, KCH * COUT)
    return np.ascontiguousarray(wm).astype(NPDT)


# ---------------------------------------------------------------- program
_PROG = None


def _build_program():
    nc = bacc.Bacc(
        "TRN2",
        target_bir_lowering=False,
        debug=False,
        enable_asserts=False,
        num_devices=NCORES,
        num_swdge_queues=NQ,
    )
    L_t = nc.dram_tensor("xbasis", [HP * WP, 4 * C], DT, kind="ExternalInput")
    WL_t = nc.dram_tensor("wmat", [128, KCH * COUT], DT, kind="ExternalInput")
    IDX_t = nc.dram_tensor("idx", [128, JBLK * NGJB * NCOLG], mybir.dt.int16,
                           kind="ExternalInput")
    WSL_t = nc.dram_tensor("wslot", [128, KK * JBLK * JC * 2], F32,
                           kind="ExternalInput")
    ID_t = nc.dram_tensor("ident", [128, 128], DT, kind="ExternalInput")
    OUT_t = nc.dram_tensor("out", [COUT, J], F32, kind="ExternalOutput")
    out_ap = OUT_t.ap()

    src_ap = bass.AP(L_t, 0, [[4 * C, HP * WP], [1, 4 * C]])

    with tile.TileContext(nc) as tc, ExitStack() as ctx:
        const = ctx.enter_context(tc.tile_pool(name="const", bufs=1))
        rpool = ctx.enter_context(tc.tile_pool(name="r", bufs=3))
        upool = ctx.enter_context(tc.tile_pool(name="u", bufs=4))
        wpool = ctx.enter_context(tc.tile_pool(name="w", bufs=4))
        vpool = ctx.enter_context(tc.tile_pool(name="v", bufs=4))
        gpool = ctx.enter_context(tc.tile_pool(name="g", bufs=2))
        rhspool = ctx.enter_context(tc.tile_pool(name="rhs", bufs=2))
        outpool = ctx.enter_context(tc.tile_pool(name="osb", bufs=2))
        pst = ctx.enter_context(tc.tile_pool(name="pst", bufs=4, space="PSUM"))
        psm = ctx.enter_context(tc.tile_pool(name="psm", bufs=4, space="PSUM"))

        # idx slices load first (per-gather) so gather 0 starts immediately;
        # the big weight matrix loads last (first needed ~40us in).
        idx_sb = const.tile([128, JBLK * NGJB * NCOLG], mybir.dt.int16)
        idx_ap = IDX_t.ap()
        for g in range(JBLK * NGJB):
            nc.scalar.dma_start(idx_sb[:, g * NCOLG:(g + 1) * NCOLG],
                                idx_ap[:, g * NCOLG:(g + 1) * NCOLG])
        wsl_sb = const.tile([128, KK * JBLK * JC * 2], F32)
        nc.sync.dma_start(wsl_sb[:], WSL_t.ap())
        id_sb = const.tile([128, 128], DT)
        nc.sync.dma_start(id_sb[:], ID_t.ap())
        w_sb = const.tile([128, KCH * COUT], DT)
        nc.sync.dma_start(w_sb[:], WL_t.ap())

        reg_full = nc.gpsimd.to_reg(NIDXG)
        reg_2jb = nc.gpsimd.to_reg(2 * JB)
        reg_1jb = nc.gpsimd.to_reg(JB)

        for jb in range(JBLK):
            gt = gpool.tile([128, JC, KK * C], DT)
            for tg in range(NGJB):
                g = jb * NGJB + tg
                last = (jb == JBLK - 1 and tg == NGJB - 1)
                if not last:
                    r = rpool.tile([128, TPG * JC, 4 * C], DT, tag="r")
                    nc.gpsimd.dma_gather(
                        r[:],
                        src_ap,
                        idx_sb[:, g * NCOLG:(g + 1) * NCOLG],
                        NIDXG,
                        reg_full,
                        4 * C,
                        queue_num=g % NQ,
                        single_packet=False,
                    )
                else:
                    # split the final gather 2/3 + 1/3 so its DMA/compute
                    # tail after the descgen train is shorter
                    r = rpool.tile([128, TPG * JC, 4 * C], DT, tag="r")
                    n1 = 2 * JB
                    c0 = g * NCOLG
                    nc.gpsimd.dma_gather(
                        r[:, 0:2 * JC, :],
                        src_ap,
                        idx_sb[:, c0:c0 + n1 // 16],
                        n1,
                        reg_2jb,
                        4 * C,
                        queue_num=g % NQ,
                        single_packet=False,
                    )
                    nc.gpsimd.dma_gather(
                        r[:, 2 * JC:3 * JC, :],
                        src_ap,
                        idx_sb[:, c0 + n1 // 16:(g + 1) * NCOLG],
                        JB,
                        reg_1jb,
                        4 * C,
                        queue_num=g % NQ,
                        single_packet=False,
                    )
                for trel in range(TPG):
                    t = tg * TPG + trel
                    for jc in range(JC):
                        slot = trel * JC + jc
                        cw = ((t * JBLK + jb) * JC + jc) * 2
                        dxs = wsl_sb[:, cw:cw + 1]
                        dys = wsl_sb[:, cw + 1:cw + 2]
                        # w = [A + dx*B, C + dx*E]  (DVE fused STT)
                        w = wpool.tile([128, 2 * C], DT, tag="w")
                        nc.vector.scalar_tensor_tensor(
                            w[:], r[:, slot, 2 * C:4 * C], dxs,
                            r[:, slot, 0:2 * C], mult, add)
                        gslc = gt[:, jc, t * C:(t + 1) * C]
                        if t == KK - 1:
                            # final tap: keep op3 on DVE (fused STT) so the
                            # pipeline tail has one less cross-engine hop
                            nc.vector.scalar_tensor_tensor(
                                gslc, w[:, C:2 * C], dys, w[:, 0:C],
                                mult, add)
                        else:
                            # v = dy * w_hi (Scalar), g = w_lo + v (DVE 2x)
                            v = vpool.tile([128, C], DT, tag="v")
                            nc.scalar.activation(v[:], w[:, C:2 * C],
                                                 ACT_COPY, scale=dys)
                            nc.vector.tensor_add(gslc, w[:, 0:C], v[:])

            rhs = rhspool.tile([128, KCH, JB], DT)
            for kc in range(KCH):
                if kc % 3 in XBAR_KC_MOD:
                    for jc in range(JC):
                        nc.sync.dma_start_transpose(
                            rhs[:, kc, jc * 128:(jc + 1) * 128],
                            gt[:, jc, kc * 128:(kc + 1) * 128])
                else:
                    ps = pst.tile([128, JB], DT)
                    for jc in range(JC):
                        nc.tensor.transpose(ps[:, jc * 128:(jc + 1) * 128],
                                            gt[:, jc, kc * 128:(kc + 1) * 128],
                                            id_sb[:])
                    nc.scalar.copy(rhs[:, kc, :], ps[:])

            for m in range(MCH):
                pso = psm.tile([128, JB], F32)
                for kc in range(KCH):
                    nc.tensor.matmul(
                        pso[:],
                        w_sb[:, kc * COUT + m * 128:kc * COUT + (m + 1) * 128],
                        rhs[:, kc, :],
                        start=(kc == 0),
                        stop=(kc == KCH - 1),
                    )
                osb = outpool.tile([128, JB], F32)
                nc.vector.tensor_copy(osb[:], pso[:])
                nc.sync.dma_start(
                    out_ap[m * 128:(m + 1) * 128, jb * JB:(jb + 1) * JB],
                    osb[:])

    nc.compile()
    return nc


def _get_program():
    global _PROG
    if _PROG is None:
        _PROG = _build_program()
    return _PROG


# ---------------------------------------------------------------- entry
def make_in_maps(x, offset, weight):
    x = np.asarray(x, np.float32)
    offset = np.asarray(offset, np.float32)
    weight = np.asarray(weight, np.float32)
    WL = _pack_weight(weight)
    ident = np.eye(128, dtype=NPDT)
    basis = [_make_basis_layout(x[b]) for b in range(B)]
    maps = []
    for core in range(NCORES):
        b, half = core // 2, core % 2
        idx, w = _make_idx_w(offset[b], half * ROWS)
        maps.append({
            "xbasis": basis[b],
            "idx": _pack_idx(idx),
            "wslot": _pack_w(w),
            "wmat": WL,
            "ident": ident,
        })
    return maps


def assemble(results):
    full = np.zeros((B, COUT, H, W), np.float32)
    for core in range(NCORES):
        b, half = core // 2, core % 2
        h0 = half * ROWS
        full[b, :, h0:h0 + ROWS, :] = \
            np.asarray(results[core]["out"]).reshape(COUT, ROWS, W)
    return full


def kernel(x, offset, weight):
    nc = _get_program()
    in_maps = make_in_maps(x, offset, weight)
    res = run_bass_kernel_spmd(nc, in_maps, list(range(NCORES)))
    return assemble(res.results)


# revision 14
# speedup vs baseline: 1.1922x; 1.0036x over previous
"""Deformable conv 3x3 (B=4, C=256, H=W=64, Cout=256) on 8 trn2 NeuronCores.

Sharding: data-parallel — core i handles batch i//2, output-row half i%2
(32 rows = 2048 output positions per core); weight replicated.

Per-core device pipeline (v3, all shapes hardcoded for this problem):
  1. Host precomputes a zero-padded "bilinear basis" image per batch:
     for each padded pixel p=(y,x): [A, C, B, E] x 256ch fp16 where
     A=x[y,x], C=x[y+1,x]-A, B=x[y,x+1]-A, E=x[y+1,x+1]-x[y+1,x]-x[y,x+1]+A.
     Bilinear sample == (A + dx*B) + dy*(C + dx*E), with zero padding
     reproducing the reference's out-of-image masking.
  2. dma_gather (SWDGE): batched — per (jb, tap-triple) gather 1536 basis
     rows (2KB each) -> R [128 j, 12, 1024] fp16, round-robin over 4
     SWDGE queues so descriptor prep overlaps transfers.
  3. Combine split across engines per (tap, jc):
       u = [B,E] * dx        (Scalar engine activation, per-partition scale)
       w = [A,C] + u         (DVE tensor_tensor, 2x fp16)
       g = w_lo + dy * w_hi  (DVE scalar_tensor_tensor, or Scalar mul +
                              DVE add for some taps to balance load)
  4. Transpose G -> rhs [c_kk, j]: PE identity-matmul transposes (PSUM,
     then Scalar copy to SBUF) for most chunks; DMA-XBAR transposes
     (sync-engine dma_start_transpose, straight to SBUF) for some chunks
     to offload the PE.
  5. GEMM: out[o, j] = sum_{c,kk} W[(kk,c), o] * rhs[(kk,c), j], fp32 PSUM,
     K = 2304 (18 chunks), M = 256 (2 chunks), N = 512 per jblock.

kernel(x, offset, weight) takes full fp32 inputs, returns [4,256,64,64] fp32.
"""
import numpy as np
from contextlib import ExitStack

import concourse.bass as bass
import concourse.bacc as bacc
import concourse.tile as tile
from concourse import mybir
from concourse.bass_utils import run_bass_kernel_spmd

# ---------------------------------------------------------------- constants
B, C, H, W = 4, 256, 64, 64
COUT = 256
K = 3
KK = 9
NCORES = 8
ROWS = 32              # output rows per core
J = ROWS * W           # 2048 output positions per core
JBLK = 4               # jblocks
JB = J // JBLK         # 512
JC = JB // 128         # 4
KCH = (C * KK) // 128  # 18 contraction chunks
MCH = COUT // 128      # 2
PADM = 8               # padding margin (covers |offset| < 7)
HP = H + 2 * PADM      # 80
WP = W + 2 * PADM      # 80

TPG = 3                # taps per gather
NGJB = KK // TPG       # gathers per jblock
NIDXG = TPG * JB       # indices per gather (1536)
NCOLG = NIDXG // 16    # idx columns per gather
NQ = 1                 # SWDGE queues

# which taps route step-3's dy-multiply through the Scalar engine
OP3_SCALAR_TAPS = frozenset((1, 4, 7))
# which (kc % 3) values route the transpose through the DMA XBAR
XBAR_KC_MOD = frozenset()

DT = mybir.dt.float16
NPDT = np.float16
F32 = mybir.dt.float32

mult = mybir.AluOpType.mult
add = mybir.AluOpType.add
ACT_COPY = mybir.ActivationFunctionType.Copy


# ---------------------------------------------------------------- host prep
def _make_basis_layout(xb):
    """xb [C,H,W] fp32 -> L [HP*WP, 4*C] fp16 basis rows [A, C, B, E]."""
    xp = np.zeros((HP, WP, C), np.float32)
    xp[PADM:PADM + H, PADM:PADM + W] = xb.transpose(1, 2, 0)
    out = np.zeros((HP, WP, 4, C), np.float32)
    a = xp[:-1, :-1]
    out[:-1, :-1, 0] = a                                # A
    out[:-1, :-1, 1] = xp[1:, :-1] - a                  # C (dy term)
    out[:-1, :-1, 2] = xp[:-1, 1:] - a                  # B (dx term)
    out[:-1, :-1, 3] = xp[1:, 1:] - xp[1:, :-1] - xp[:-1, 1:] + a  # E
    return out.reshape(HP * WP, 4 * C).astype(NPDT)


def _make_idx_w(offset_b, h0):
    """-> idx [KK, J] int16 (padded-grid row), w [KK, J, 2] fp32 (dx, dy)."""
    off = offset_b.reshape(KK, 2, H, W)
    ho = np.arange(h0, h0 + ROWS, dtype=np.float32)
    wo = np.arange(W, dtype=np.float32)
    ky = np.repeat(np.arange(K, dtype=np.float32), K)
    kx = np.tile(np.arange(K, dtype=np.float32), K)
    py = ho[None, :, None] + ky[:, None, None] - 1.0 + off[:, 0, h0:h0 + ROWS, :]
    px = wo[None, None, :] + kx[:, None, None] - 1.0 + off[:, 1, h0:h0 + ROWS, :]
    y0f = np.floor(py)
    x0f = np.floor(px)
    dy = (py - y0f).astype(np.float32)
    dx = (px - x0f).astype(np.float32)
    yi = np.clip(y0f.astype(np.int64) + PADM, 0, HP - 2)
    xi = np.clip(x0f.astype(np.int64) + PADM, 0, WP - 2)
    idx = (yi * WP + xi).astype(np.int16)
    w = np.stack([dx, dy], axis=-1)
    return idx.reshape(KK, J), w.reshape(KK, J, 2)


def _pack_idx(idx):
    """[KK, J] -> [128, JBLK*NGJB*NCOLG] int16; gather g=(jb, tg) covers
    taps tg*TPG..+TPG over jblock jb; idx i at [i%16, i//16] within the
    gather's NCOLG-column slice, replicated to 8 groups of 16 partitions."""
    out = np.zeros((16, JBLK * NGJB * NCOLG), np.int16)
    for jb in range(JBLK):
        for tg in range(NGJB):
            g = jb * NGJB + tg
            v = idx[tg * TPG:(tg + 1) * TPG, jb * JB:(jb + 1) * JB].reshape(-1)
            out[:, g * NCOLG:(g + 1) * NCOLG] = v.reshape(NCOLG, 16).T
    return np.tile(out, (8, 1))


def _pack_w(w):
    """[KK, J, 2] -> [128, KK*JBLK*JC*2] fp32; col ((t*JBLK+jb)*JC+jc)*2+s."""
    a = w.reshape(KK, JBLK, JC, 128, 2)
    return np.ascontiguousarray(
        a.transpose(3, 0, 1, 2, 4).reshape(128, KK * JBLK * JC * 2))


def _pack_weight(weight):
    """[COUT, C, 3, 3] fp32 -> [128, KCH*COUT] fp16; K-order kk*C+c,
    lhsT tile (kc, m) at cols [kc*COUT + m*128, +128)."""
    wm = weight.reshape(COUT, C, KK).transpose(2, 1, 0).reshape(KK * C, COUT)
    wm = wm.reshape(KCH, 128, COUT).transpose(1, 0, 2).reshape(128, KCH * COUT)
    return np.ascontiguousarray(wm).astype(NPDT)


# ---------------------------------------------------------------- program
_PROG = None


def _build_program():
    nc = bacc.Bacc(
        "TRN2",
        target_bir_lowering=False,
        debug=False,
        enable_asserts=False,
        num_devices=NCORES,
        num_swdge_queues=NQ,
    )
    L_t = nc.dram_tensor("xbasis", [HP * WP, 4 * C], DT, kind="ExternalInput")
    WL_t = nc.dram_tensor("wmat", [128, KCH * COUT], DT, kind="ExternalInput")
    IDX_t = nc.dram_tensor("idx", [128, JBLK * NGJB * NCOLG], mybir.dt.int16,
                           kind="ExternalInput")
    WSL_t = nc.dram_tensor("wslot", [128, KK * JBLK * JC * 2], F32,
                           kind="ExternalInput")
    ID_t = nc.dram_tensor("ident", [128, 128], DT, kind="ExternalInput")
    OUT_t = nc.dram_tensor("out", [COUT, J], DT, kind="ExternalOutput")
    out_ap = OUT_t.ap()

    src_ap = bass.AP(L_t, 0, [[4 * C, HP * WP], [1, 4 * C]])

    with tile.TileContext(nc) as tc, ExitStack() as ctx:
        const = ctx.enter_context(tc.tile_pool(name="const", bufs=1))
        rpool = ctx.enter_context(tc.tile_pool(name="r", bufs=3))
        upool = ctx.enter_context(tc.tile_pool(name="u", bufs=4))
        wpool = ctx.enter_context(tc.tile_pool(name="w", bufs=4))
        vpool = ctx.enter_context(tc.tile_pool(name="v", bufs=4))
        gpool = ctx.enter_context(tc.tile_pool(name="g", bufs=2))
        rhspool = ctx.enter_context(tc.tile_pool(name="rhs", bufs=2))
        outpool = ctx.enter_context(tc.tile_pool(name="osb", bufs=2))
        pst = ctx.enter_context(tc.tile_pool(name="pst", bufs=4, space="PSUM"))
        psm = ctx.enter_context(tc.tile_pool(name="psm", bufs=4, space="PSUM"))

        # idx slices load first (per-gather) so gather 0 starts immediately;
        # the big weight matrix loads last (first needed ~40us in).
        idx_sb = const.tile([128, JBLK * NGJB * NCOLG], mybir.dt.int16)
        idx_ap = IDX_t.ap()
        for g in range(JBLK * NGJB):
            nc.scalar.dma_start(idx_sb[:, g * NCOLG:(g + 1) * NCOLG],
                                idx_ap[:, g * NCOLG:(g + 1) * NCOLG])
        wsl_sb = const.tile([128, KK * JBLK * JC * 2], F32)
        nc.sync.dma_start(wsl_sb[:], WSL_t.ap())
        id_sb = const.tile([128, 128], DT)
        nc.sync.dma_start(id_sb[:], ID_t.ap())
        w_sb = const.tile([128, KCH * COUT], DT)
        nc.sync.dma_start(w_sb[:], WL_t.ap())

        reg_full = nc.gpsimd.to_reg(NIDXG)
        reg_2jb = nc.gpsimd.to_reg(2 * JB)
        reg_1jb = nc.gpsimd.to_reg(JB)

        for jb in range(JBLK):
            gt = gpool.tile([128, JC, KK * C], DT)
            for tg in range(NGJB):
                g = jb * NGJB + tg
                last = (jb == JBLK - 1 and tg == NGJB - 1)
                if not last:
                    r = rpool.tile([128, TPG * JC, 4 * C], DT, tag="r")
                    nc.gpsimd.dma_gather(
                        r[:],
                        src_ap,
                        idx_sb[:, g * NCOLG:(g + 1) * NCOLG],
                        NIDXG,
                        reg_full,
                        4 * C,
                        queue_num=g % NQ,
                        single_packet=False,
                    )
                else:
                    # split the final gather 2/3 + 1/3 so its DMA/compute
                    # tail after the descgen train is shorter
                    r = rpool.tile([128, TPG * JC, 4 * C], DT, tag="r")
                    n1 = 2 * JB
                    c0 = g * NCOLG
                    nc.gpsimd.dma_gather(
                        r[:, 0:2 * JC, :],
                        src_ap,
                        idx_sb[:, c0:c0 + n1 // 16],
                        n1,
                        reg_2jb,
                        4 * C,
                        queue_num=g % NQ,
                        single_packet=False,
                    )
                    nc.gpsimd.dma_gather(
                        r[:, 2 * JC:3 * JC, :],
                        src_ap,
                        idx_sb[:, c0 + n1 // 16:(g + 1) * NCOLG],
                        JB,
                        reg_1jb,
                        4 * C,
                        queue_num=g % NQ,
                        single_packet=False,
                    )
                for trel in range(TPG):
                    t = tg * TPG + trel
                    for jc in range(JC):
                        slot = trel * JC + jc
                        cw = ((t * JBLK + jb) * JC + jc) * 2
                        dxs = wsl_sb[:, cw:cw + 1]
                        dys = wsl_sb[:, cw + 1:cw + 2]
                        # w = [A + dx*B, C + dx*E]  (DVE fused STT)
                        w = wpool.tile([128, 2 * C], DT, tag="w")
                        nc.vector.scalar_tensor_tensor(
                            w[:], r[:, slot, 2 * C:4 * C], dxs,
                            r[:, slot, 0:2 * C], mult, add)
                        gslc = gt[:, jc, t * C:(t + 1) * C]
                        if t == KK - 1:
                            # final tap: keep op3 on DVE (fused STT) so the
                            # pipeline tail has one less cross-engine hop
                            nc.vector.scalar_tensor_tensor(
                                gslc, w[:, C:2 * C], dys, w[:, 0:C],
                                mult, add)
                        else:
                            # v = dy * w_hi (Scalar), g = w_lo + v (DVE 2x)
                            v = vpool.tile([128, C], DT, tag="v")
                            nc.scalar.activation(v[:], w[:, C:2 * C],
                                                 ACT_COPY, scale=dys)
                            nc.vector.tensor_add(gslc, w[:, 0:C], v[:])

            rhs = rhspool.tile([128, KCH, JB], DT)
            for kc in range(KCH):
                if kc % 3 in XBAR_KC_MOD:
                    for jc in range(JC):
                        nc.sync.dma_start_transpose(
                            rhs[:, kc, jc * 128:(jc + 1) * 128],
                            gt[:, jc, kc * 128:(kc + 1) * 128])
                else:
                    ps = pst.tile([128, JB], DT)
                    for jc in range(JC):
                        nc.tensor.transpose(ps[:, jc * 128:(jc + 1) * 128],
                                            gt[:, jc, kc * 128:(kc + 1) * 128],
                                            id_sb[:])
                    nc.scalar.copy(rhs[:, kc, :], ps[:])

            for m in range(MCH):
                pso = psm.tile([128, JB], F32)
                for kc in range(KCH):
                    nc.tensor.matmul(
                        pso[:],
                        w_sb[:, kc * COUT + m * 128:kc * COUT + (m + 1) * 128],
                        rhs[:, kc, :],
                        start=(kc == 0),
                        stop=(kc == KCH - 1),
                    )
                osb = outpool.tile([128, JB], DT)
                nc.vector.tensor_copy(osb[:], pso[:])
                nc.sync.dma_start(
                    out_ap[m * 128:(m + 1) * 128, jb * JB:(jb + 1) * JB],
                    osb[:])

    nc.compile()
    return nc


def _get_program():
    global _PROG
    if _PROG is None:
        _PROG = _build_program()
    return _PROG


# ---------------------------------------------------------------- entry
def make_in_maps(x, offset, weight):
    x = np.asarray(x, np.float32)
    offset = np.asarray(offset, np.float32)
    weight = np.asarray(weight, np.float32)
    WL = _pack_weight(weight)
    ident = np.eye(128, dtype=NPDT)
    basis = [_make_basis_layout(x[b]) for b in range(B)]
    maps = []
    for core in range(NCORES):
        b, half = core // 2, core % 2
        idx, w = _make_idx_w(offset[b], half * ROWS)
        maps.append({
            "xbasis": basis[b],
            "idx": _pack_idx(idx),
            "wslot": _pack_w(w),
            "wmat": WL,
            "ident": ident,
        })
    return maps


def assemble(results):
    full = np.zeros((B, COUT, H, W), np.float32)
    for core in range(NCORES):
        b, half = core // 2, core % 2
        h0 = half * ROWS
        full[b, :, h0:h0 + ROWS, :] = np.asarray(
            results[core]["out"]).astype(np.float32).reshape(COUT, ROWS, W)
    return full


def kernel(x, offset, weight):
    nc = _get_program()
    in_maps = make_in_maps(x, offset, weight)
    res = run_bass_kernel_spmd(nc, in_maps, list(range(NCORES)))
    return assemble(res.results)


# revision 17
# speedup vs baseline: 1.1985x; 1.0053x over previous
"""Deformable conv 3x3 (B=4, C=256, H=W=64, Cout=256) on 8 trn2 NeuronCores.

Sharding: data-parallel — core i handles batch i//2, output-row half i%2
(32 rows = 2048 output positions per core); weight replicated.

Per-core device pipeline (v3, all shapes hardcoded for this problem):
  1. Host precomputes a zero-padded "bilinear basis" image per batch:
     for each padded pixel p=(y,x): [A, C, B, E] x 256ch fp16 where
     A=x[y,x], C=x[y+1,x]-A, B=x[y,x+1]-A, E=x[y+1,x+1]-x[y+1,x]-x[y,x+1]+A.
     Bilinear sample == (A + dx*B) + dy*(C + dx*E), with zero padding
     reproducing the reference's out-of-image masking.
  2. dma_gather (SWDGE): batched — per (jb, tap-triple) gather 1536 basis
     rows (2KB each) -> R [128 j, 12, 1024] fp16, round-robin over 4
     SWDGE queues so descriptor prep overlaps transfers.
  3. Combine split across engines per (tap, jc):
       u = [B,E] * dx        (Scalar engine activation, per-partition scale)
       w = [A,C] + u         (DVE tensor_tensor, 2x fp16)
       g = w_lo + dy * w_hi  (DVE scalar_tensor_tensor, or Scalar mul +
                              DVE add for some taps to balance load)
  4. Transpose G -> rhs [c_kk, j]: PE identity-matmul transposes (PSUM,
     then Scalar copy to SBUF) for most chunks; DMA-XBAR transposes
     (sync-engine dma_start_transpose, straight to SBUF) for some chunks
     to offload the PE.
  5. GEMM: out[o, j] = sum_{c,kk} W[(kk,c), o] * rhs[(kk,c), j], fp32 PSUM,
     K = 2304 (18 chunks), M = 256 (2 chunks), N = 512 per jblock.

kernel(x, offset, weight) takes full fp32 inputs, returns [4,256,64,64] fp32.
"""
import numpy as np
from contextlib import ExitStack

import concourse.bass as bass
import concourse.bacc as bacc
import concourse.tile as tile
from concourse import mybir
from concourse.bass_utils import run_bass_kernel_spmd

# ---------------------------------------------------------------- constants
B, C, H, W = 4, 256, 64, 64
COUT = 256
K = 3
KK = 9
NCORES = 8
ROWS = 32              # output rows per core
J = ROWS * W           # 2048 output positions per core
JBLK = 4               # jblocks
JB = J // JBLK         # 512
JC = JB // 128         # 4
KCH = (C * KK) // 128  # 18 contraction chunks
MCH = COUT // 128      # 2
PADM = 8               # padding margin (covers |offset| < 7)
HP = H + 2 * PADM      # 80
WP = W + 2 * PADM      # 80

TPG = 3                # taps per gather
NGJB = KK // TPG       # gathers per jblock
NIDXG = TPG * JB       # indices per gather (1536)
NCOLG = NIDXG // 16    # idx columns per gather
NQ = 1                 # SWDGE queues

# which taps route step-3's dy-multiply through the Scalar engine
OP3_SCALAR_TAPS = frozenset((1, 4, 7))
# which (kc % 3) values route the transpose through the DMA XBAR
XBAR_KC_MOD = frozenset()

DT = mybir.dt.float16
NPDT = np.float16
F32 = mybir.dt.float32

mult = mybir.AluOpType.mult
add = mybir.AluOpType.add
ACT_COPY = mybir.ActivationFunctionType.Copy


# ---------------------------------------------------------------- host prep
def _make_basis_layout(xb):
    """xb [C,H,W] fp32 -> L [HP*WP, 4*C] fp16 basis rows [A, C, B, E]."""
    xp = np.zeros((HP, WP, C), np.float32)
    xp[PADM:PADM + H, PADM:PADM + W] = xb.transpose(1, 2, 0)
    out = np.zeros((HP, WP, 4, C), np.float32)
    a = xp[:-1, :-1]
    out[:-1, :-1, 0] = a                                # A
    out[:-1, :-1, 1] = xp[1:, :-1] - a                  # C (dy term)
    out[:-1, :-1, 2] = xp[:-1, 1:] - a                  # B (dx term)
    out[:-1, :-1, 3] = xp[1:, 1:] - xp[1:, :-1] - xp[:-1, 1:] + a  # E
    return out.reshape(HP * WP, 4 * C).astype(NPDT)


def _make_idx_w(offset_b, h0):
    """-> idx [KK, J] int16 (padded-grid row), w [KK, J, 2] fp32 (dx, dy)."""
    off = offset_b.reshape(KK, 2, H, W)
    ho = np.arange(h0, h0 + ROWS, dtype=np.float32)
    wo = np.arange(W, dtype=np.float32)
    ky = np.repeat(np.arange(K, dtype=np.float32), K)
    kx = np.tile(np.arange(K, dtype=np.float32), K)
    py = ho[None, :, None] + ky[:, None, None] - 1.0 + off[:, 0, h0:h0 + ROWS, :]
    px = wo[None, None, :] + kx[:, None, None] - 1.0 + off[:, 1, h0:h0 + ROWS, :]
    y0f = np.floor(py)
    x0f = np.floor(px)
    dy = (py - y0f).astype(np.float32)
    dx = (px - x0f).astype(np.float32)
    yi = np.clip(y0f.astype(np.int64) + PADM, 0, HP - 2)
    xi = np.clip(x0f.astype(np.int64) + PADM, 0, WP - 2)
    idx = (yi * WP + xi).astype(np.int16)
    w = np.stack([dx, dy], axis=-1)
    return idx.reshape(KK, J), w.reshape(KK, J, 2)


def _pack_idx(idx):
    """[KK, J] -> [128, JBLK*NGJB*NCOLG] int16; gather g=(jb, tg) covers
    taps tg*TPG..+TPG over jblock jb; idx i at [i%16, i//16] within the
    gather's NCOLG-column slice, replicated to 8 groups of 16 partitions."""
    out = np.zeros((16, JBLK * NGJB * NCOLG), np.int16)
    for jb in range(JBLK):
        for tg in range(NGJB):
            g = jb * NGJB + tg
            v = idx[tg * TPG:(tg + 1) * TPG, jb * JB:(jb + 1) * JB].reshape(-1)
            out[:, g * NCOLG:(g + 1) * NCOLG] = v.reshape(NCOLG, 16).T
    return np.tile(out, (8, 1))


def _pack_w(w):
    """[KK, J, 2] -> [128, KK*JBLK*JC*2] fp32; col ((t*JBLK+jb)*JC+jc)*2+s."""
    a = w.reshape(KK, JBLK, JC, 128, 2)
    return np.ascontiguousarray(
        a.transpose(3, 0, 1, 2, 4).reshape(128, KK * JBLK * JC * 2))


def _pack_weight(weight):
    """[COUT, C, 3, 3] fp32 -> [128, KCH*COUT] fp16; K-order kk*C+c,
    lhsT tile (kc, m) at cols [kc*COUT + m*128, +128)."""
    wm = weight.reshape(COUT, C, KK).transpose(2, 1, 0).reshape(KK * C, COUT)
    wm = wm.reshape(KCH, 128, COUT).transpose(1, 0, 2).reshape(128, KCH * COUT)
    return np.ascontiguousarray(wm).astype(NPDT)


# ---------------------------------------------------------------- program
_PROG = None


def _build_program():
    nc = bacc.Bacc(
        "TRN2",
        target_bir_lowering=False,
        debug=False,
        enable_asserts=False,
        num_devices=NCORES,
        num_swdge_queues=NQ,
    )
    L_t = nc.dram_tensor("xbasis", [HP * WP, 4 * C], DT, kind="ExternalInput")
    WL_t = nc.dram_tensor("wmat", [128, KCH * COUT], DT, kind="ExternalInput")
    IDX_t = nc.dram_tensor("idx", [128, JBLK * NGJB * NCOLG], mybir.dt.int16,
                           kind="ExternalInput")
    WSL_t = nc.dram_tensor("wslot", [128, KK * JBLK * JC * 2], F32,
                           kind="ExternalInput")
    ID_t = nc.dram_tensor("ident", [128, 128], DT, kind="ExternalInput")
    OUT_t = nc.dram_tensor("out", [COUT, J], DT, kind="ExternalOutput")
    out_ap = OUT_t.ap()

    src_ap = bass.AP(L_t, 0, [[4 * C, HP * WP], [1, 4 * C]])

    with tile.TileContext(nc) as tc, ExitStack() as ctx:
        const = ctx.enter_context(tc.tile_pool(name="const", bufs=1))
        rpool = ctx.enter_context(tc.tile_pool(name="r", bufs=3))
        upool = ctx.enter_context(tc.tile_pool(name="u", bufs=2))
        wpool = ctx.enter_context(tc.tile_pool(name="w", bufs=4))
        vpool = ctx.enter_context(tc.tile_pool(name="v", bufs=4))
        gpool = ctx.enter_context(tc.tile_pool(name="g", bufs=2))
        rhspool = ctx.enter_context(tc.tile_pool(name="rhs", bufs=2))
        outpool = ctx.enter_context(tc.tile_pool(name="osb", bufs=2))
        pst = ctx.enter_context(tc.tile_pool(name="pst", bufs=4, space="PSUM"))
        psm = ctx.enter_context(tc.tile_pool(name="psm", bufs=4, space="PSUM"))

        # idx slices load first (per-gather) so gather 0 starts immediately;
        # the big weight matrix loads last (first needed ~40us in).
        idx_sb = const.tile([128, JBLK * NGJB * NCOLG], mybir.dt.int16)
        idx_ap = IDX_t.ap()
        for g in range(JBLK * NGJB):
            nc.scalar.dma_start(idx_sb[:, g * NCOLG:(g + 1) * NCOLG],
                                idx_ap[:, g * NCOLG:(g + 1) * NCOLG])
        wsl_sb = const.tile([128, KK * JBLK * JC * 2], F32)
        nc.sync.dma_start(wsl_sb[:], WSL_t.ap())
        id_sb = const.tile([128, 128], DT)
        nc.sync.dma_start(id_sb[:], ID_t.ap())
        w_sb = const.tile([128, KCH * COUT], DT)
        nc.sync.dma_start(w_sb[:], WL_t.ap())

        reg_full = nc.gpsimd.to_reg(NIDXG)
        reg_2jb = nc.gpsimd.to_reg(2 * JB)
        reg_1jb = nc.gpsimd.to_reg(JB)

        for jb in range(JBLK):
            gt = gpool.tile([128, JC, KK * C], DT)
            for tg in range(NGJB):
                g = jb * NGJB + tg
                last = (jb == JBLK - 1 and tg == NGJB - 1)
                if not last:
                    r = rpool.tile([128, TPG * JC, 4 * C], DT, tag="r")
                    nc.gpsimd.dma_gather(
                        r[:],
                        src_ap,
                        idx_sb[:, g * NCOLG:(g + 1) * NCOLG],
                        NIDXG,
                        reg_full,
                        4 * C,
                        queue_num=g % NQ,
                        single_packet=False,
                    )
                else:
                    # split the final gather 2/3 + 1/3 so its DMA/compute
                    # tail after the descgen train is shorter
                    r = rpool.tile([128, TPG * JC, 4 * C], DT, tag="r")
                    n1 = 2 * JB
                    c0 = g * NCOLG
                    nc.gpsimd.dma_gather(
                        r[:, 0:2 * JC, :],
                        src_ap,
                        idx_sb[:, c0:c0 + n1 // 16],
                        n1,
                        reg_2jb,
                        4 * C,
                        queue_num=g % NQ,
                        single_packet=False,
                    )
                    nc.gpsimd.dma_gather(
                        r[:, 2 * JC:3 * JC, :],
                        src_ap,
                        idx_sb[:, c0 + n1 // 16:(g + 1) * NCOLG],
                        JB,
                        reg_1jb,
                        4 * C,
                        queue_num=g % NQ,
                        single_packet=False,
                    )
                for trel in range(TPG):
                    t = tg * TPG + trel
                    for jc in range(JC):
                        slot = trel * JC + jc
                        cw = ((t * JBLK + jb) * JC + jc) * 2
                        dxs = wsl_sb[:, cw:cw + 1]
                        dys = wsl_sb[:, cw + 1:cw + 2]
                        gslc = gt[:, jc, t * C:(t + 1) * C]
                        tail = (jb == JBLK - 1 and t >= 2 * TPG)
                        if tail and (t + jc) % 2 == 1:
                            # tail ACT form: both multiplies on the Scalar
                            # engine, adds on DVE — spreads the last
                            # jblock's combine across both engines
                            u = upool.tile([128, 2 * C], DT, tag="u")
                            nc.scalar.activation(
                                u[:], r[:, slot, 2 * C:4 * C], ACT_COPY,
                                scale=dxs)
                            w = wpool.tile([128, 2 * C], DT, tag="w")
                            nc.vector.tensor_add(w[:], r[:, slot, 0:2 * C],
                                                 u[:])
                            v = vpool.tile([128, C], DT, tag="v")
                            nc.scalar.activation(v[:], w[:, C:2 * C],
                                                 ACT_COPY, scale=dys)
                            nc.vector.tensor_add(gslc, w[:, 0:C], v[:])
                        else:
                            # w = [A + dx*B, C + dx*E]  (DVE fused STT)
                            w = wpool.tile([128, 2 * C], DT, tag="w")
                            nc.vector.scalar_tensor_tensor(
                                w[:], r[:, slot, 2 * C:4 * C], dxs,
                                r[:, slot, 0:2 * C], mult, add)
                            if tail or t == KK - 1:
                                # keep the chain on DVE (fused STT)
                                nc.vector.scalar_tensor_tensor(
                                    gslc, w[:, C:2 * C], dys, w[:, 0:C],
                                    mult, add)
                            else:
                                # v = dy*w_hi (Scalar), g = w_lo+v (DVE)
                                v = vpool.tile([128, C], DT, tag="v")
                                nc.scalar.activation(v[:], w[:, C:2 * C],
                                                     ACT_COPY, scale=dys)
                                nc.vector.tensor_add(gslc, w[:, 0:C], v[:])

            rhs = rhspool.tile([128, KCH, JB], DT)
            for kc in range(KCH):
                if kc % 3 in XBAR_KC_MOD:
                    for jc in range(JC):
                        nc.sync.dma_start_transpose(
                            rhs[:, kc, jc * 128:(jc + 1) * 128],
                            gt[:, jc, kc * 128:(kc + 1) * 128])
                else:
                    ps = pst.tile([128, JB], DT)
                    for jc in range(JC):
                        nc.tensor.transpose(ps[:, jc * 128:(jc + 1) * 128],
                                            gt[:, jc, kc * 128:(kc + 1) * 128],
                                            id_sb[:])
                    nc.scalar.copy(rhs[:, kc, :], ps[:])

            for m in range(MCH):
                pso = psm.tile([128, JB], F32)
                for kc in range(KCH):
                    nc.tensor.matmul(
                        pso[:],
                        w_sb[:, kc * COUT + m * 128:kc * COUT + (m + 1) * 128],
                        rhs[:, kc, :],
                        start=(kc == 0),
                        stop=(kc == KCH - 1),
                    )
                osb = outpool.tile([128, JB], DT)
                nc.vector.tensor_copy(osb[:], pso[:])
                nc.sync.dma_start(
                    out_ap[m * 128:(m + 1) * 128, jb * JB:(jb + 1) * JB],
                    osb[:])

    nc.compile()
    return nc


def _get_program():
    global _PROG
    if _PROG is None:
        _PROG = _build_program()
    return _PROG


# ---------------------------------------------------------------- entry
def make_in_maps(x, offset, weight):
    x = np.asarray(x, np.float32)
    offset = np.asarray(offset, np.float32)
    weight = np.asarray(weight, np.float32)
    WL = _pack_weight(weight)
    ident = np.eye(128, dtype=NPDT)
    basis = [_make_basis_layout(x[b]) for b in range(B)]
    maps = []
    for core in range(NCORES):
        b, half = core // 2, core % 2
        idx, w = _make_idx_w(offset[b], half * ROWS)
        maps.append({
            "xbasis": basis[b],
            "idx": _pack_idx(idx),
            "wslot": _pack_w(w),
            "wmat": WL,
            "ident": ident,
        })
    return maps


def assemble(results):
    full = np.zeros((B, COUT, H, W), np.float32)
    for core in range(NCORES):
        b, half = core // 2, core % 2
        h0 = half * ROWS
        full[b, :, h0:h0 + ROWS, :] = np.asarray(
            results[core]["out"]).astype(np.float32).reshape(COUT, ROWS, W)
    return full


def kernel(x, offset, weight):
    nc = _get_program()
    in_maps = make_in_maps(x, offset, weight)
    res = run_bass_kernel_spmd(nc, in_maps, list(range(NCORES)))
    return assemble(res.results)
